# revision 1
# baseline (speedup 1.0000x reference)
"""KAN encoder (2 KAN layers + relu + linear head) on 8 trn2 NeuronCores.

Strategy: data-parallel on batch (512 rows/core), all weights replicated.
Layout is feature-on-partition / batch-on-free throughout, so no device-side
transposes are needed. The spline path is a dense matmul over (in*8) with the
B-spline bases computed on the ACT+DVE engines via the exact identity

    bases_k(x) * 6 = relu(2 - |u-k-2|)^3 - 4*relu(1 - |u-k-2|)^3,
    u = (x + 2.2) / 0.4

(the 1/6 is folded into the spline weights host-side, as is the spline_scaler).
All matmuls run in float32r (tf32-class precision, 1 cycle/row at N=512).
"""
import numpy as np
from contextlib import ExitStack

from concourse import bacc, tile, mybir
from concourse.bass_utils import run_bass_kernel_spmd

F32 = mybir.dt.float32
F32R = mybir.dt.float32r
AF = mybir.ActivationFunctionType

B, D_IN, H0, H1, L = 4096, 1024, 2048, 1024, 512
NCORES = 8
BC = B // NCORES          # 512 batch cols per core
NB = 512                  # free-dim (batch) tile = full per-core batch
CBRT4 = float(4.0 ** (1.0 / 3.0))

_cache = {}


def _build_full():
    nc = bacc.Bacc("TRN2", target_bir_lowering=False, debug=False,
                   num_devices=NCORES)

    x_d = nc.dram_tensor("x_d", [8, 128, BC], F32, kind="ExternalInput")
    w0_d = nc.dram_tensor("w0_d", [8, 128, 9, H0], F32R, kind="ExternalInput")
    w1_d = nc.dram_tensor("w1_d", [16, 128, 9, H1], F32R, kind="ExternalInput")
    dw_d = nc.dram_tensor("dw_d", [8, 128, L], F32R, kind="ExternalInput")
    db_d = nc.dram_tensor("db_d", [128, 4], F32, kind="ExternalInput")
    o_d = nc.dram_tensor("o_d", [4, 128, BC], F32, kind="ExternalOutput")

    with tile.TileContext(nc) as tc, ExitStack() as ctx:
        cpool = ctx.enter_context(tc.tile_pool(name="cpool", bufs=1))
        psum = ctx.enter_context(tc.tile_pool(name="psum", bufs=1, space="PSUM"))
        h0p = ctx.enter_context(tc.tile_pool(name="h0p", bufs=1))
        ab = ctx.enter_context(tc.tile_pool(name="ab", bufs=1))

        bias_tiles = {}

        def bias_ap(val):
            val = float(val)
            if val not in bias_tiles:
                t = cpool.tile([128, 1], F32, tag=f"b{len(bias_tiles)}",
                               name=f"bias{len(bias_tiles)}")
                nc.gpsimd.memset(t[:, :], val)
                bias_tiles[val] = t
            return bias_tiles[val][:, :]

        h0_sb = h0p.tile([128, 16 * NB], F32, name="h0_sb")

        def emit_feats(src_ap, feats, tagp):
            """feats (128, 9*NB) f32: j=0 silu(src); j=1+k -> bases_k(src)*6."""
            nc.scalar.activation(feats[:, 0:NB], src_ap, AF.Silu,
                                 bias=bias_ap(0.0), scale=1.0)
            for k in range(8):
                s = ab.tile([128, NB], F32, tag="s", bufs=2, name=f"s{tagp}_{k}")
                a = ab.tile([128, NB], F32, tag="a", bufs=2, name=f"a{tagp}_{k}")
                e = ab.tile([128, NB], F32, tag="e", bufs=2, name=f"e{tagp}_{k}")
                a2 = ab.tile([128, NB], F32, tag="a2", bufs=2, name=f"a2{tagp}_{k}")
                a3 = ab.tile([128, NB], F32, tag="a3", bufs=2, name=f"a3{tagp}_{k}")
                e2 = ab.tile([128, NB], F32, tag="e2", bufs=2, name=f"e2{tagp}_{k}")
                e3 = ab.tile([128, NB], F32, tag="e3", bufs=2, name=f"e3{tagp}_{k}")
                nc.scalar.activation(s[:, :], src_ap, AF.Abs,
                                     bias=bias_ap(3.5 - k), scale=2.5)
                nc.scalar.activation(a[:, :], s[:, :], AF.Relu,
                                     bias=bias_ap(2.0), scale=-1.0)
                nc.scalar.activation(e[:, :], s[:, :], AF.Relu,
                                     bias=bias_ap(CBRT4), scale=-CBRT4)
                nc.vector.tensor_mul(a2[:, :], a[:, :], a[:, :])
                nc.vector.tensor_mul(a3[:, :], a2[:, :], a[:, :])
                if k % 3 != 0:
                    nc.scalar.activation(e2[:, :], e[:, :], AF.Square,
                                         bias=bias_ap(0.0), scale=1.0)
                else:
                    nc.vector.tensor_mul(e2[:, :], e[:, :], e[:, :])
                nc.vector.tensor_mul(e3[:, :], e2[:, :], e[:, :])
                nc.vector.tensor_sub(feats[:, (k + 1) * NB:(k + 2) * NB],
                                     a3[:, :], e3[:, :])

        # ---- Layer 0: out chunks in two groups of 8 PSUM banks ----
        for og in range(2):
            pts = [psum.tile([128, NB], F32, tag=f"bank{oc}",
                             name=f"psA{og}_{oc}") for oc in range(8)]
            for ic in range(8):
                xt = ab.tile([128, NB], F32, tag="xt", bufs=2, name=f"xt{og}_{ic}")
                nc.sync.dma_start(out=xt[:, :], in_=x_d[ic, :, :])
                feats = ab.tile([128, 9 * NB], F32R, tag="feats", bufs=2,
                                name=f"f0_{og}_{ic}")
                emit_feats(xt[:, :], feats, f"0_{og}_{ic}")
                wsl = ab.tile([128, 9, 8, 128], F32R, tag="wsl", bufs=2,
                              name=f"w0_{og}_{ic}")
                nc.sync.dma_start(out=wsl[:, :, :, :],
                                  in_=w0_d[ic, :, :, og * 1024:(og + 1) * 1024])
                for oc in range(8):
                    for j in range(9):
                        nc.tensor.matmul(
                            pts[oc][:, :], wsl[:, j, oc, :],
                            feats[:, j * NB:(j + 1) * NB],
                            start=(ic == 0 and j == 0),
                            stop=(ic == 7 and j == 8))
            for oc in range(8):
                nc.scalar.activation(
                    h0_sb[:, (og * 8 + oc) * NB:(og * 8 + oc + 1) * NB],
                    pts[oc][:, :], AF.Copy, bias=0.0, scale=1.0)

        # ---- Layer 1: 8 out chunks, 16 contraction chunks ----
        pts = [psum.tile([128, NB], F32, tag=f"bank{oc}", name=f"psB{oc}")
               for oc in range(8)]
        for ic in range(16):
            feats = ab.tile([128, 9 * NB], F32R, tag="feats", bufs=2,
                            name=f"f1_{ic}")
            emit_feats(h0_sb[:, ic * NB:(ic + 1) * NB], feats, f"1_{ic}")
            wsl = ab.tile([128, 9, 8, 128], F32R, tag="wsl", bufs=2,
                          name=f"w1_{ic}")
            nc.sync.dma_start(out=wsl[:, :, :, :], in_=w1_d[ic, :, :, :])
            for oc in range(8):
                for j in range(9):
                    nc.tensor.matmul(
                        pts[oc][:, :], wsl[:, j, oc, :],
                        feats[:, j * NB:(j + 1) * NB],
                        start=(ic == 0 and j == 0),
                        stop=(ic == 15 and j == 8))

        # ---- Head: relu(h1) @ dw.T + db ----
        rl = ab.tile([128, 8 * NB], F32R, tag="feats", bufs=2, name="rl")
        for oc in range(8):
            nc.scalar.activation(rl[:, oc * NB:(oc + 1) * NB],
                                 pts[oc][:, :], AF.Relu,
                                 bias=bias_ap(0.0), scale=1.0)
        dwt = ab.tile([128, 8, L], F32R, tag="wsl", bufs=2, name="dwt")
        for ic in range(8):
            nc.sync.dma_start(out=dwt[:, ic, :], in_=dw_d[ic, :, :])
        dbt = cpool.tile([128, 4], F32, name="dbt")
        nc.sync.dma_start(out=dbt[:, :], in_=db_d[:, :])
        out_sb = ab.tile([128, 4 * NB], F32, tag="outsb", name="out_sb")
        for lc in range(4):
            pt = psum.tile([128, NB], F32, tag=f"bank{lc}", name=f"psC{lc}")
            for ic in range(8):
                nc.tensor.matmul(pt[:, :], dwt[:, ic, lc * 128:(lc + 1) * 128],
                                 rl[:, ic * NB:(ic + 1) * NB],
                                 start=(ic == 0), stop=(ic == 7))
            nc.scalar.activation(out_sb[:, lc * NB:(lc + 1) * NB], pt[:, :],
                                 AF.Identity, bias=dbt[:, lc:lc + 1], scale=1.0)
            nc.sync.dma_start(out=o_d[lc, :, :],
                              in_=out_sb[:, lc * NB:(lc + 1) * NB])

    nc.compile()
    return nc


def _prep_weights(bw0, sw0, ss0, bw1, sw1, ss1, dw, db):
    # layer0: (8 ic, 128 r, 9 j, 2048 o); j=0 -> bw0.T, j=1+k -> (sw0*ss0/6).T
    w0 = np.empty((8, 128, 9, H0), np.float32)
    w0[:, :, 0, :] = bw0.T.reshape(8, 128, H0)
    s0 = (sw0 * (ss0[:, :, None] / 6.0)).transpose(1, 2, 0)  # (i, k, o)
    w0[:, :, 1:, :] = s0.reshape(8, 128, 8, H0)
    w1 = np.empty((16, 128, 9, H1), np.float32)
    w1[:, :, 0, :] = bw1.T.reshape(16, 128, H1)
    s1 = (sw1 * (ss1[:, :, None] / 6.0)).transpose(1, 2, 0)
    w1[:, :, 1:, :] = s1.reshape(16, 128, 8, H1)
    dwt = np.ascontiguousarray(dw.T.reshape(8, 128, L))
    dbt = np.ascontiguousarray(db.reshape(4, 128).T)
    return (np.ascontiguousarray(w0), np.ascontiguousarray(w1), dwt, dbt)


def kernel(x, bw0, sw0, ss0, bw1, sw1, ss1, dw, db):
    if "nc" not in _cache:
        _cache["nc"] = _build_full()
    nc = _cache["nc"]
    w0, w1, dwt, dbt = _prep_weights(
        np.asarray(bw0, np.float32), np.asarray(sw0, np.float32),
        np.asarray(ss0, np.float32), np.asarray(bw1, np.float32),
        np.asarray(sw1, np.float32), np.asarray(ss1, np.float32),
        np.asarray(dw, np.float32), np.asarray(db, np.float32))
    xT = np.ascontiguousarray(np.asarray(x, np.float32).T)  # (1024, 4096)
    in_maps = []
    for c in range(NCORES):
        xc = np.ascontiguousarray(
            xT[:, c * BC:(c + 1) * BC].reshape(8, 128, BC))
        in_maps.append({"x_d": xc, "w0_d": w0, "w1_d": w1,
                        "dw_d": dwt, "db_d": dbt})
    _cache["in_maps"] = in_maps
    res = run_bass_kernel_spmd(nc, in_maps, list(range(NCORES)))
    out = np.empty((B, L), np.float32)
    for c in range(NCORES):
        oc = res.results[c]["o_d"]          # (4, 128, BC)
        out[c * BC:(c + 1) * BC, :] = oc.reshape(L, BC).T
    return out



# revision 4
# speedup vs baseline: 2.2052x; 2.2052x over previous
"""KAN encoder (2 KAN layers + relu + linear head) on 8 trn2 NeuronCores.

Data-parallel on batch (512 rows/core), weights replicated. Feature-on-
partition layout throughout (no device-side transposes).

Fast path vs the f32r baseline:
  * spline matmuls run in fp8 (e4m3) DoubleRow mode: 2 contraction slots
    per PE pass -> 2x tensor throughput; base/silu path runs in fp16.
  * the 8 B-spline basis functions are evaluated as a super-Gaussian
        bases6(t)/8 ~= C * exp(-A t^2 - B t^4)   (max abs err 1.5e-3)
    which needs only 8 small tensor_scalar ops + 2 big DVE ops + 1 big
    ACT Exp per 128x512 chunk (vs ~60 elementwise ops exact).
  * weights ship as fp8/fp16 (4x less HBM traffic than f32).
  * PSUM carries 64*h; the 1/64 folds into the evac copy (layer0) or the
    head weights (layer1), so evacuation is a single ACT op per bank.
"""
import numpy as np
from contextlib import ExitStack

from concourse import bacc, tile, mybir
from concourse.bass_utils import run_bass_kernel_spmd

F32 = mybir.dt.float32
F16 = mybir.dt.float16
F8 = mybir.dt.float8e4
AF = mybir.ActivationFunctionType
ALU = mybir.AluOpType
DR = mybir.MatmulPerfMode.DoubleRow

B, D_IN, H0, H1, L = 4096, 1024, 2048, 1024, 512
NCORES = 8
BC = B // NCORES          # 512 batch cols per core
NB = 512                  # free-dim tile = full per-core batch

# super-Gaussian fit of the cardinal cubic B-spline (t in knot units):
#   bases6(t)/8 ~= CFIT * exp(-AFIT t^2 - BFIT t^4)
CFIT, AFIT, BFIT = 0.49848316, 1.2742171, 0.11364197
LNC = float(np.log(CFIT))
SCALE = 64.0              # PSUM carries SCALE * h

_cache = {}


def _build():
    nc = bacc.Bacc("TRN2", target_bir_lowering=False, debug=False,
                   num_devices=NCORES)

    x_d = nc.dram_tensor("x_d", [8, 128, BC], F32, kind="ExternalInput")
    w0s_d = nc.dram_tensor("w0s_d", [8, 128, 8, H0], F8, kind="ExternalInput")
    w0b_d = nc.dram_tensor("w0b_d", [8, 128, H0], F16, kind="ExternalInput")
    w1s_d = nc.dram_tensor("w1s_d", [16, 128, 8, H1], F8, kind="ExternalInput")
    w1b_d = nc.dram_tensor("w1b_d", [16, 128, H1], F16, kind="ExternalInput")
    dw_d = nc.dram_tensor("dw_d", [8, 128, L], F16, kind="ExternalInput")
    db_d = nc.dram_tensor("db_d", [128, 4], F32, kind="ExternalInput")
    o_d = nc.dram_tensor("o_d", [4, 128, BC], F32, kind="ExternalOutput")

    with tile.TileContext(nc) as tc, ExitStack() as ctx:
        cpool = ctx.enter_context(tc.tile_pool(name="cpool", bufs=1))
        psum = ctx.enter_context(tc.tile_pool(name="psum", bufs=1, space="PSUM"))
        fix = ctx.enter_context(tc.tile_pool(name="fix", bufs=1))
        sp = ctx.enter_context(tc.tile_pool(name="sp", bufs=1))
        dp = ctx.enter_context(tc.tile_pool(name="dp", bufs=2))

        dbt = cpool.tile([128, 4], F32, name="dbt")
        nc.sync.dma_start(out=dbt[:, :], in_=db_d[:, :])

        bias_tiles = {}

        def bias_ap(val):
            val = float(val)
            if val not in bias_tiles:
                t = cpool.tile([128, 1], F32, tag=f"b{len(bias_tiles)}",
                               name=f"bias{len(bias_tiles)}")
                nc.gpsimd.memset(t[:, :], val)
                bias_tiles[val] = t
            return bias_tiles[val][:, :]
        dwt = fix.tile([128, 8, L], F16, name="dwt")
        for i in range(8):
            nc.sync.dma_start(out=dwt[:, i, :], in_=dw_d[i, :, :])

        def emit_feats(src16, fdst, sdst, tag):
            """src16 [128,NB] fp16 -> fdst [128,8,NB] fp8 bases, sdst silu."""
            nc.scalar.activation(sdst, src16, AF.Silu, bias=bias_ap(0.0), scale=1.0)
            t = sp.tile([128, 8, NB], F16, tag="t", name=f"t{tag}")
            for k in range(8):
                mu = 0.4 * k - 1.4
                nc.vector.tensor_scalar(t[:, k, :], src16, -mu, 2.5,
                                        ALU.add, ALU.mult)
            t2 = sp.tile([128, 8, NB], F16, tag="t2", name=f"t2{tag}")
            nc.vector.tensor_tensor(t2[:, :, :], t[:, :, :], t[:, :, :],
                                    ALU.mult)
            xx = dp.tile([128, 8, NB], F16, tag="xx", bufs=2, name=f"xx{tag}")
            nc.vector.scalar_tensor_tensor(xx[:, :, :], t2[:, :, :],
                                           AFIT / BFIT, t2[:, :, :],
                                           ALU.add, ALU.mult)
            nc.scalar.activation(fdst, xx[:, :, :], AF.Exp,
                                 bias=bias_ap(LNC), scale=-BFIT)

        # ---- layer 0 feats: all 8 input chunks, computed once & cached ----
        f0 = [fix.tile([128, 8, NB], F8, name=f"f0_{ic}") for ic in range(8)]
        s0 = [fix.tile([128, NB], F16, name=f"s0_{ic}") for ic in range(8)]
        for ic in range(8):
            xt = dp.tile([128, NB], F32, tag="xt", bufs=2, name=f"xt{ic}")
            nc.sync.dma_start(out=xt[:, :], in_=x_d[ic, :, :])
            x16 = dp.tile([128, NB], F16, tag="x16", bufs=2, name=f"x16_{ic}")
            nc.scalar.activation(x16[:, :], xt[:, :], AF.Copy,
                                 bias=0.0, scale=1.0)
            emit_feats(x16[:, :], f0[ic][:, :, :], s0[ic][:, :], f"a{ic}")

        h0 = [fix.tile([128, NB], F16, name=f"h0_{j}") for j in range(16)]

        # ---- layer 0 matmuls: two output groups of 8 PSUM banks ----
        for og in range(2):
            pts = [psum.tile([128, NB], F32, tag=f"bank{oc}",
                             name=f"psA{og}_{oc}") for oc in range(8)]
            for ic in range(8):
                ws = dp.tile([128, 8, 1024], F8, tag="ws", bufs=2,
                             name=f"w0s{og}_{ic}")
                nc.sync.dma_start(out=ws[:, :, :],
                                  in_=w0s_d[ic, :, :, og * 1024:(og + 1) * 1024])
                wb = dp.tile([128, 1024], F16, tag="wb", bufs=2,
                             name=f"w0b{og}_{ic}")
                nc.sync.dma_start(out=wb[:, :],
                                  in_=w0b_d[ic, :, og * 1024:(og + 1) * 1024])
                for oc in range(8):
                    nc.tensor.matmul(pts[oc][:, :],
                                     wb[:, oc * 128:(oc + 1) * 128],
                                     s0[ic][:, :],
                                     start=(ic == 0), stop=False)
                    for j in range(4):
                        nc.tensor.matmul(
                            pts[oc][:, :],
                            ws[:, 2 * j:2 * j + 2, oc * 128:(oc + 1) * 128],
                            f0[ic][:, 2 * j:2 * j + 2, :],
                            start=False, stop=(ic == 7 and j == 3),
                            perf_mode=DR)
            for oc in range(8):
                nc.scalar.activation(h0[og * 8 + oc][:, :], pts[oc][:, :],
                                     AF.Copy, bias=0.0, scale=1.0 / SCALE)

        # ---- layer 1: 16 contraction chunks into 8 PSUM banks ----
        pts = [psum.tile([128, NB], F32, tag=f"bank{oc}", name=f"psB{oc}")
               for oc in range(8)]
        for ic in range(16):
            f1 = dp.tile([128, 8, NB], F8, tag="f1", bufs=2, name=f"f1_{ic}")
            s1 = dp.tile([128, NB], F16, tag="s1", bufs=2, name=f"s1_{ic}")
            emit_feats(h0[ic][:, :], f1[:, :, :], s1[:, :], f"b{ic}")
            ws = dp.tile([128, 8, 1024], F8, tag="ws", bufs=2, name=f"w1s{ic}")
            nc.sync.dma_start(out=ws[:, :, :], in_=w1s_d[ic, :, :, :])
            wb = dp.tile([128, 1024], F16, tag="wb", bufs=2, name=f"w1b{ic}")
            nc.sync.dma_start(out=wb[:, :], in_=w1b_d[ic, :, :])
            for oc in range(8):
                nc.tensor.matmul(pts[oc][:, :], wb[:, oc * 128:(oc + 1) * 128],
                                 s1[:, :], start=(ic == 0), stop=False)
                for j in range(4):
                    nc.tensor.matmul(
                        pts[oc][:, :],
                        ws[:, 2 * j:2 * j + 2, oc * 128:(oc + 1) * 128],
                        f1[:, 2 * j:2 * j + 2, :],
                        start=False, stop=(ic == 15 and j == 3),
                        perf_mode=DR)

        # ---- head: out = relu(h1) @ (dw/SCALE).T + db ----
        rl = fix.tile([128, 8, NB], F16, name="rl")
        for oc in range(8):
            nc.scalar.activation(rl[:, oc, :], pts[oc][:, :], AF.Relu,
                                 bias=0.0, scale=1.0)
        out_sb = fix.tile([128, 4, NB], F32, name="out_sb")
        for lc in range(4):
            pt = psum.tile([128, NB], F32, tag=f"bank{lc}", name=f"psC{lc}")
            for i8 in range(8):
                nc.tensor.matmul(pt[:, :],
                                 dwt[:, i8, lc * 128:(lc + 1) * 128],
                                 rl[:, i8, :],
                                 start=(i8 == 0), stop=(i8 == 7))
            nc.scalar.activation(out_sb[:, lc, :], pt[:, :], AF.Identity,
                                 bias=dbt[:, lc:lc + 1], scale=1.0)
            nc.sync.dma_start(out=o_d[lc, :, :], in_=out_sb[:, lc, :])

    nc.compile()
    return nc


def _prep_weights(bw0, sw0, ss0, bw1, sw1, ss1, dw, db):
    e4 = mybir.dt.np(F8)
    f16 = np.float16
    w0s = (sw0 * ss0[:, :, None] * (SCALE / 0.75)).transpose(1, 2, 0)
    w0s = np.ascontiguousarray(w0s.reshape(8, 128, 8, H0)).astype(e4)
    w0b = np.ascontiguousarray((bw0 * SCALE).T.reshape(8, 128, H0)).astype(f16)
    w1s = (sw1 * ss1[:, :, None] * (SCALE / 0.75)).transpose(1, 2, 0)
    w1s = np.ascontiguousarray(w1s.reshape(16, 128, 8, H1)).astype(e4)
    w1b = np.ascontiguousarray((bw1 * SCALE).T.reshape(16, 128, H1)).astype(f16)
    dwt = np.ascontiguousarray((dw / SCALE).T.reshape(8, 128, L)).astype(f16)
    dbt = np.ascontiguousarray(db.reshape(4, 128).T.astype(np.float32))
    return w0s, w0b, w1s, w1b, dwt, dbt


def kernel(x, bw0, sw0, ss0, bw1, sw1, ss1, dw, db):
    if "nc" not in _cache:
        _cache["nc"] = _build()
    nc = _cache["nc"]
    w0s, w0b, w1s, w1b, dwt, dbt = _prep_weights(
        np.asarray(bw0, np.float32), np.asarray(sw0, np.float32),
        np.asarray(ss0, np.float32), np.asarray(bw1, np.float32),
        np.asarray(sw1, np.float32), np.asarray(ss1, np.float32),
        np.asarray(dw, np.float32), np.asarray(db, np.float32))
    xT = np.ascontiguousarray(np.asarray(x, np.float32).T)  # (1024, 4096)
    in_maps = []
    for c in range(NCORES):
        xc = np.ascontiguousarray(
            xT[:, c * BC:(c + 1) * BC].reshape(8, 128, BC))
        in_maps.append({"x_d": xc, "w0s_d": w0s, "w0b_d": w0b,
                        "w1s_d": w1s, "w1b_d": w1b,
                        "dw_d": dwt, "db_d": dbt})
    _cache["in_maps"] = in_maps
    res = run_bass_kernel_spmd(nc, in_maps, list(range(NCORES)))
    out = np.empty((B, L), np.float32)
    for c in range(NCORES):
        oc = res.results[c]["o_d"]          # (4, 128, BC)
        out[c * BC:(c + 1) * BC, :] = oc.reshape(L, BC).T
    return out


# revision 6
# speedup vs baseline: 2.2069x; 1.0008x over previous
"""KAN encoder (2 KAN layers + relu + linear head) on 8 trn2 NeuronCores.

Data-parallel on batch (512 rows/core), weights replicated. Feature-on-
partition layout throughout (no device-side transposes).

Fast path vs the f32r baseline:
  * spline matmuls run in fp8 (e4m3) DoubleRow mode: 256 contraction rows
    per instruction vs 128 -> ~2x fewer PE instructions; base path fp16.
  * the 8 B-spline basis functions are evaluated as a super-Gaussian
        bases6(t)/8 ~= C * exp(-A t^2 - B t^4)   (max abs err 1.5e-3)
    which needs only 8 small tensor_scalar ops + 2 big DVE ops + 1 big
    ACT Exp per 128x512 chunk (vs ~60 elementwise ops exact).
  * weights ship as fp8/fp16 (4x less HBM traffic than f32).
  * PSUM carries 64*h; the 1/64 folds into the evac copy (layer0) or the
    head weights (layer1), so evacuation is a single ACT op per bank.
  * per output group, all fp16 base matmuls are issued before the fp8
    DoubleRow matmuls: the base run gives the PE runway while the
    DVE/ACT pipeline produces the fp8 basis features.
  * ACT ops are batched by function (Silu blocks, Exp runs, evac Copy
    blocks) to avoid activation-table reload thrash; layer-1 feats for
    h0 chunks 0-7 are emitted between the two layer-0 output groups so
    they overlap the og1 matmul phase.
  * weights stream on the SP DMA queue; x / head weights / outputs use
    the GPSIMD queue so the first weight tile lands immediately.
"""
import numpy as np
from contextlib import ExitStack

from concourse import bacc, tile, mybir
from concourse.bass_utils import run_bass_kernel_spmd

F32 = mybir.dt.float32
F16 = mybir.dt.float16
F8 = mybir.dt.float8e4
AF = mybir.ActivationFunctionType
ALU = mybir.AluOpType
DR = mybir.MatmulPerfMode.DoubleRow

B, D_IN, H0, H1, L = 4096, 1024, 2048, 1024, 512
NCORES = 8
BC = B // NCORES          # 512 batch cols per core
NB = 512                  # free-dim tile = full per-core batch

# super-Gaussian fit of the cardinal cubic B-spline (t in knot units):
#   bases6(t)/8 ~= CFIT * exp(-AFIT t^2 - BFIT t^4)
CFIT, AFIT, BFIT = 0.49848316, 1.2742171, 0.11364197
LNC = float(np.log(CFIT))
SCALE = 64.0              # PSUM carries SCALE * h

_cache = {}


def _build():
    nc = bacc.Bacc("TRN2", target_bir_lowering=False, debug=False,
                   num_devices=NCORES)

    x_d = nc.dram_tensor("x_d", [8, 128, BC], F32, kind="ExternalInput")
    w0s_d = nc.dram_tensor("w0s_d", [8, 128, 8, H0], F8, kind="ExternalInput")
    w0b_d = nc.dram_tensor("w0b_d", [8, 128, H0], F16, kind="ExternalInput")
    w1s_d = nc.dram_tensor("w1s_d", [16, 128, 8, H1], F8, kind="ExternalInput")
    w1b_d = nc.dram_tensor("w1b_d", [16, 128, H1], F16, kind="ExternalInput")
    dw_d = nc.dram_tensor("dw_d", [8, 128, L], F16, kind="ExternalInput")
    db_d = nc.dram_tensor("db_d", [128, 4], F32, kind="ExternalInput")
    o_d = nc.dram_tensor("o_d", [4, 128, BC], F32, kind="ExternalOutput")

    with tile.TileContext(nc) as tc, ExitStack() as ctx:
        cpool = ctx.enter_context(tc.tile_pool(name="cpool", bufs=1))
        psum = ctx.enter_context(tc.tile_pool(name="psum", bufs=1, space="PSUM"))
        fix = ctx.enter_context(tc.tile_pool(name="fix", bufs=1))
        sp = ctx.enter_context(tc.tile_pool(name="sp", bufs=1))
        dp = ctx.enter_context(tc.tile_pool(name="dp", bufs=2))

        bias_tiles = {}

        def bias_ap(val):
            val = float(val)
            if val not in bias_tiles:
                t = cpool.tile([128, 1], F32, tag=f"b{len(bias_tiles)}",
                               name=f"bias{len(bias_tiles)}")
                nc.gpsimd.memset(t[:, :], val)
                bias_tiles[val] = t
            return bias_tiles[val][:, :]

        bias_ap(0.0)
        bias_ap(LNC)

        # ---- input / head-weight / bias DMAs on the gpsimd queue ----
        xts = []
        for ic in range(8):
            xt = dp.tile([128, NB], F32, tag="xt", bufs=3, name=f"xt{ic}")
            nc.gpsimd.dma_start(out=xt[:, :], in_=x_d[ic, :, :])
            xts.append(xt)
        dbt = cpool.tile([128, 4], F32, name="dbt")
        nc.gpsimd.dma_start(out=dbt[:, :], in_=db_d[:, :])
        dwt = fix.tile([128, 8, L], F16, name="dwt")
        for i in range(8):
            nc.gpsimd.dma_start(out=dwt[:, i, :], in_=dw_d[i, :, :])

        # ---- layer-0 og0 base weights first on the SP queue ----
        def load_wb0(og):
            tiles = []
            for ic in range(8):
                wb = dp.tile([128, 1024], F16, tag="wb", bufs=4,
                             name=f"w0b{og}_{ic}")
                nc.sync.dma_start(out=wb[:, :],
                                  in_=w0b_d[ic, :, og * 1024:(og + 1) * 1024])
                tiles.append(wb)
            return tiles

        wb_og0 = load_wb0(0)

        def dve_chain(src16, fdst, tag):
            """src16 [128,NB] fp16 -> fdst [128,8,NB] fp8 basis features."""
            t = sp.tile([128, 8, NB], F16, tag="t", name=f"t{tag}")
            for k in range(8):
                mu = 0.4 * k - 1.4
                nc.vector.tensor_scalar(t[:, k, :], src16, -mu, 2.5,
                                        ALU.add, ALU.mult)
            t2 = sp.tile([128, 8, NB], F16, tag="t2", name=f"t2{tag}")
            nc.vector.tensor_tensor(t2[:, :, :], t[:, :, :], t[:, :, :],
                                    ALU.mult)
            xx = dp.tile([128, 8, NB], F16, tag="xx", bufs=2, name=f"xx{tag}")
            nc.vector.scalar_tensor_tensor(xx[:, :, :], t2[:, :, :],
                                           AFIT / BFIT, t2[:, :, :],
                                           ALU.add, ALU.mult)
            nc.scalar.activation(fdst, xx[:, :, :], AF.Exp,
                                 bias=bias_ap(LNC), scale=-BFIT)

        # feats tiles: f0[0..7] and f1[0..15] share one rotating pool slot
        # set; f1[i] starts reusing f0 slots once og1's DR run has read them
        def feat_tile(name):
            return dp.tile([128, 8, NB], F8, tag="fx", bufs=12, name=name)

        # ---- layer 0 feats: casts (DVE), silu block (ACT), then chains ----
        s0 = [fix.tile([128, NB], F16, name=f"s0_{ic}") for ic in range(8)]
        x16s = []
        for ic in range(8):
            x16 = dp.tile([128, NB], F16, tag="x16", bufs=8, name=f"x16_{ic}")
            nc.vector.tensor_scalar(x16[:, :], xts[ic][:, :], 0.0, None,
                                    ALU.add)
            x16s.append(x16)
        for ic in range(8):
            nc.scalar.activation(s0[ic][:, :], x16s[ic][:, :], AF.Silu,
                                 bias=bias_ap(0.0), scale=1.0)
        f0 = []
        for ic in range(8):
            f = feat_tile(f"f0_{ic}")
            dve_chain(x16s[ic][:, :], f[:, :, :], f"a{ic}")
            f0.append(f)

        h0 = [fix.tile([128, NB], F16, name=f"h0_{j}") for j in range(16)]
        s1 = [fix.tile([128, NB], F16, name=f"s1_{j}") for j in range(16)]
        f1 = [None] * 16

        def emit_l1_feats(half):
            for ic in range(half * 8, half * 8 + 8):
                nc.scalar.activation(s1[ic][:, :], h0[ic][:, :], AF.Silu,
                                     bias=bias_ap(0.0), scale=1.0)
            for ic in range(half * 8, half * 8 + 8):
                f = feat_tile(f"f1_{ic}")
                dve_chain(h0[ic][:, :], f[:, :, :], f"b{ic}")
                f1[ic] = f

        # ---- layer 0 matmuls: base run first, then DoubleRow run ----
        for og in range(2):
            wb = wb_og0 if og == 0 else load_wb0(1)
            pts = [psum.tile([128, NB], F32, tag=f"bank{oc}",
                             name=f"psA{og}_{oc}") for oc in range(8)]
            for ic in range(8):
                for oc in range(8):
                    nc.tensor.matmul(pts[oc][:, :],
                                     wb[ic][:, oc * 128:(oc + 1) * 128],
                                     s0[ic][:, :],
                                     start=(ic == 0), stop=False)
            for ic in range(8):
                ws = dp.tile([128, 8, 1024], F8, tag="ws", bufs=3,
                             name=f"w0s{og}_{ic}")
                nc.sync.dma_start(out=ws[:, :, :],
                                  in_=w0s_d[ic, :, :, og * 1024:(og + 1) * 1024])
                for oc in range(8):
                    for j in range(4):
                        nc.tensor.matmul(
                            pts[oc][:, :],
                            ws[:, 2 * j:2 * j + 2, oc * 128:(oc + 1) * 128],
                            f0[ic][:, 2 * j:2 * j + 2, :],
                            start=False, stop=(ic == 7 and j == 3),
                            perf_mode=DR)
            for oc in range(8):
                nc.scalar.activation(h0[og * 8 + oc][:, :], pts[oc][:, :],
                                     AF.Copy, bias=0.0, scale=1.0 / SCALE)
            # layer-1 feats for the h0 chunks this og group just produced;
            # for og 0 these overlap og1's matmul phase
            emit_l1_feats(og)

        # ---- layer 1 matmuls: base run, then DoubleRow run ----
        wb1 = []
        for ic in range(16):
            w = dp.tile([128, 1024], F16, tag="wb", bufs=4, name=f"w1b{ic}")
            nc.sync.dma_start(out=w[:, :], in_=w1b_d[ic, :, :])
            wb1.append(w)
        pts = [psum.tile([128, NB], F32, tag=f"bank{oc}", name=f"psB{oc}")
               for oc in range(8)]
        for ic in range(16):
            for oc in range(8):
                nc.tensor.matmul(pts[oc][:, :],
                                 wb1[ic][:, oc * 128:(oc + 1) * 128],
                                 s1[ic][:, :], start=(ic == 0), stop=False)
        for ic in range(16):
            ws = dp.tile([128, 8, 1024], F8, tag="ws", bufs=3, name=f"w1s{ic}")
            nc.sync.dma_start(out=ws[:, :, :], in_=w1s_d[ic, :, :, :])
            for oc in range(8):
                for j in range(4):
                    nc.tensor.matmul(
                        pts[oc][:, :],
                        ws[:, 2 * j:2 * j + 2, oc * 128:(oc + 1) * 128],
                        f1[ic][:, 2 * j:2 * j + 2, :],
                        start=False, stop=(ic == 15 and j == 3),
                        perf_mode=DR)

        # ---- head: out = relu(h1) @ (dw/SCALE).T + db ----
        rl = fix.tile([128, 8, NB], F16, name="rl")
        for oc in range(8):
            nc.scalar.activation(rl[:, oc, :], pts[oc][:, :], AF.Relu,
                                 bias=bias_ap(0.0), scale=1.0)
        out_sb = fix.tile([128, 4, NB], F32, name="out_sb")
        for lc in range(4):
            pt = psum.tile([128, NB], F32, tag=f"bank{lc}", name=f"psC{lc}")
            for i8 in range(8):
                nc.tensor.matmul(pt[:, :],
                                 dwt[:, i8, lc * 128:(lc + 1) * 128],
                                 rl[:, i8, :],
                                 start=(i8 == 0), stop=(i8 == 7))
            nc.scalar.activation(out_sb[:, lc, :], pt[:, :], AF.Identity,
                                 bias=dbt[:, lc:lc + 1], scale=1.0)
            nc.gpsimd.dma_start(out=o_d[lc, :, :], in_=out_sb[:, lc, :])

    nc.compile()
    return nc


def _prep_weights(bw0, sw0, ss0, bw1, sw1, ss1, dw, db):
    e4 = mybir.dt.np(F8)
    f16 = np.float16
    w0s = (sw0 * ss0[:, :, None] * (SCALE / 0.75)).transpose(1, 2, 0)
    w0s = np.ascontiguousarray(w0s.reshape(8, 128, 8, H0)).astype(e4)
    w0b = np.ascontiguousarray((bw0 * SCALE).T.reshape(8, 128, H0)).astype(f16)
    w1s = (sw1 * ss1[:, :, None] * (SCALE / 0.75)).transpose(1, 2, 0)
    w1s = np.ascontiguousarray(w1s.reshape(16, 128, 8, H1)).astype(e4)
    w1b = np.ascontiguousarray((bw1 * SCALE).T.reshape(16, 128, H1)).astype(f16)
    dwt = np.ascontiguousarray((dw / SCALE).T.reshape(8, 128, L)).astype(f16)
    dbt = np.ascontiguousarray(db.reshape(4, 128).T.astype(np.float32))
    return w0s, w0b, w1s, w1b, dwt, dbt


def kernel(x, bw0, sw0, ss0, bw1, sw1, ss1, dw, db):
    if "nc" not in _cache:
        _cache["nc"] = _build()
    nc = _cache["nc"]
    w0s, w0b, w1s, w1b, dwt, dbt = _prep_weights(
        np.asarray(bw0, np.float32), np.asarray(sw0, np.float32),
        np.asarray(ss0, np.float32), np.asarray(bw1, np.float32),
        np.asarray(sw1, np.float32), np.asarray(ss1, np.float32),
        np.asarray(dw, np.float32), np.asarray(db, np.float32))
    xT = np.ascontiguousarray(np.asarray(x, np.float32).T)  # (1024, 4096)
    in_maps = []
    for c in range(NCORES):
        xc = np.ascontiguousarray(
            xT[:, c * BC:(c + 1) * BC].reshape(8, 128, BC))
        in_maps.append({"x_d": xc, "w0s_d": w0s, "w0b_d": w0b,
                        "w1s_d": w1s, "w1b_d": w1b,
                        "dw_d": dwt, "db_d": dbt})
    _cache["in_maps"] = in_maps
    res = run_bass_kernel_spmd(nc, in_maps, list(range(NCORES)))
    out = np.empty((B, L), np.float32)
    for c in range(NCORES):
        oc = res.results[c]["o_d"]          # (4, 128, BC)
        out[c * BC:(c + 1) * BC, :] = oc.reshape(L, BC).T
    return out
